# revision 49
# baseline (speedup 1.0000x reference)
"""Multi-head cross-attention (post-LN) Trainium2 Bass kernel.

Full inputs -> full outputs. Sharding: 8 cores = 4 batches x 2 query-row
halves (512 rows each).  Host pre-transposes h/c/weights, scales weights by
32 and casts to fp8e4m3 so every big matmul runs in fp8 DoubleRow mode
(2 MACs/cell/cycle).  Scale folding keeps everything consistent:

  wq,wk,wv,wo scaled x32  ->  q~ = 32q, k~ = 32k, s~ = 1024 s
  exp scale = SCALE/1024      (softmax invariant)
  v~ = 32v -> av~ = 32 attn_vec (normalized by the ones-column denominator)
  psum_O = av~ @ 32Wo = 1024 attn_out;  hres = 1024 h  (LN is scale-inv,
  eps scaled by 1024^2)

Per-core pipeline (flat 64-chunk software pipeline, ACT-bound):
  Q proj (fp8 DR) -> q~ bf16 [e,i]
  K proj per pair (fp8 DR, filler-interleaved) -> kT bf16 [dh,j]
  V proj (fp8 DR, filler-interleaved) -> v~ fp8 in SBUF [j, jt, head, 80]
      (ones column at d=64 gives softmax denominators for free)
  scores: bf16 K=64 row-packed pairs -> PSUM [j,i]; exp on ACT -> p fp8
  attn@V: fp8 DR over j-tile pairs, accumulate [65, i] (row 64 = denom)
  normalize via PE ones-broadcast of 1/denom; O proj fp8 DR; residual+LN
"""

import sys

for _p in ("/opt/trn_rl_repo", "/root/.axon_site/_ro/trn_rl_repo"):
    if _p not in sys.path:
        sys.path.append(_p)

import numpy as np

import concourse.bass as bass
import concourse.tile as tile
from concourse import bacc, mybir
from concourse.bass_utils import run_bass_kernel_spmd

P = 128
D = 1024          # d_model
I = 512           # query rows per core
J = 2048          # kv length
NH = 16           # heads
DH = 64           # head dim
DHP = 80          # padded per-head stride in the SBUF V tile (16B aligned)
SCALE = 1.0 / (DH ** 0.5)
WS = 32.0         # weight pre-scale (weights ~N(0, 1/32) -> ~N(0,1) in fp8)
QK_SCALE = SCALE / (WS * WS)
EXP_SHIFT = -4.0  # global exp shift: keeps p in fp8e4m3 range (max ~150);
                  # softmax-invariant because the denominator uses the same p
RES_SCALE = WS * WS                      # hres = 1024*h host-side
EPS_ADJ = 1e-5 * RES_SCALE * RES_SCALE   # LN eps in the scaled domain
F32 = mybir.dt.float32
F32R = mybir.dt.float32r
BF16 = mybir.dt.bfloat16
F8 = mybir.dt.float8e4
DR = mybir.MatmulPerfMode.DoubleRow

MT = D // P       # 8 m-tiles (contraction over d_model)
ET = D // P       # 8 e-tiles (head features)
JT = J // P       # 16 j-tiles
NPAIR = NH // 2   # 8 head pairs
NCH = 8           # chunks per pair (2 j-tiles each)


def build_program(reps=1, trivial_ln_affine=False):
    nc = bacc.Bacc(None, target_bir_lowering=False, debug=False)

    hT8 = nc.dram_tensor("hT8", [D, I], F8, kind="ExternalInput")
    cT8 = nc.dram_tensor("cT8", [D, J], F8, kind="ExternalInput")
    wq8 = nc.dram_tensor("wq8", [D, D], F8, kind="ExternalInput")
    wk8 = nc.dram_tensor("wk8", [D, D], F8, kind="ExternalInput")
    wv8 = nc.dram_tensor("wv8", [D, D], F8, kind="ExternalInput")
    wo8 = nc.dram_tensor("wo8", [D, D], F8, kind="ExternalInput")
    hres = nc.dram_tensor("hres", [I, D], BF16, kind="ExternalInput")
    ident = nc.dram_tensor("ident", [P, P], BF16, kind="ExternalInput")
    gamma = nc.dram_tensor("gamma", [P, D], F32, kind="ExternalInput")
    beta = nc.dram_tensor("beta", [P, D], F32, kind="ExternalInput")
    out = nc.dram_tensor("out", [I, D], F32, kind="ExternalOutput")

    with tile.TileContext(nc) as tc:
        with (
            tc.tile_pool(name="consts", bufs=1) as consts,
            tc.tile_pool(name="persist", bufs=1) as persist,
            tc.tile_pool(name="work", bufs=2) as work,
            tc.tile_pool(name="ph4", bufs=2) as ph4,
            tc.tile_pool(name="psum", bufs=1, space="PSUM") as psum,
        ):
            gamma_bc = consts.tile([P, D], F32, tag="gamma_bc")
            beta_bc = consts.tile([P, D], F32, tag="beta_bc")
            ones_row = consts.tile([1, DH], F32R, tag="ones_row")
            nc.vector.memset(ones_row.bitcast(F32), 1.0)
            eps_t = consts.tile([P, 1], F32, tag="eps")
            nc.vector.memset(eps_t, EPS_ADJ)
            shift_t = consts.tile([P, 1], F32, tag="shift")
            nc.vector.memset(shift_t, EXP_SHIFT)

            hT8s = persist.tile([P, MT, I], F8, tag="hT8s")
            cT8s = persist.tile([P, MT, J], F8, tag="cT8s")
            wq8s = persist.tile([P, MT, D], F8, tag="wq8s")
            wk8s = persist.tile([P, MT, D], F8, tag="wk8s")
            wv8s = persist.tile([P, MT, D], F8, tag="wv8s")
            wo8s = persist.tile([P, ET, D], F8, tag="wo8s")
            qT = persist.tile([P, ET, I], BF16, tag="qT")
            v8 = persist.tile([P, JT, NH, DHP], F8, tag="v8")
            avT = persist.tile([P, ET, I], F8, tag="avT")

            # softmax-denominator ones column (constant across reps)
            nc.vector.memset(v8[:, :, :, DH : DH + 1], 1.0)

            for _rep in range(reps):
                # ---- input DMA: one big transfer per tensor (the per-call
                # issue overhead is ~1.2us, so small per-slice DMAs would
                # serialize into a huge head), K-path inputs first ---------
                nc.sync.dma_start(
                    wk8s[:, :, :],
                    wk8.ap().rearrange("(mt p) e -> p mt e", p=P))
                nc.sync.dma_start(
                    cT8s[:, :, 0 : J // 2],
                    cT8.ap()[:, 0 : J // 2]
                    .rearrange("(mt p) j -> p mt j", p=P))
                nc.sync.dma_start(
                    cT8s[:, :, J // 2 : J],
                    cT8.ap()[:, J // 2 : J]
                    .rearrange("(mt p) j -> p mt j", p=P))
                nc.sync.dma_start(
                    wq8s[:, :, :],
                    wq8.ap().rearrange("(mt p) e -> p mt e", p=P))
                nc.sync.dma_start(
                    hT8s[:, :, :],
                    hT8.ap().rearrange("(mt p) i -> p mt i", p=P))
                nc.sync.dma_start(
                    wv8s[:, :, :],
                    wv8.ap().rearrange("(mt p) e -> p mt e", p=P))
                nc.sync.dma_start(
                    wo8s[:, :, :],
                    wo8.ap().rearrange("(et p) o -> p et o", p=P))
                hres_all = ph4.tile([P, I // P, D], BF16, tag="hres", bufs=1)
                nc.sync.dma_start(
                    hres_all,
                    hres.ap().rearrange("(it p) d -> p it d", p=P))
                hres_ts = [hres_all[:, it, :] for it in range(I // P)]
                ident_t = consts.tile([P, P], BF16, tag="ident")
                nc.sync.dma_start(ident_t, ident.ap())
                if not trivial_ln_affine:
                    nc.sync.dma_start(gamma_bc, gamma.ap())
                    nc.sync.dma_start(beta_bc, beta.ap())

                # ---- Q projection (fp8 DoubleRow) ---------------------
                def q_step(et):
                    qps = psum.tile([P, I], F32, tag="kps", name="qps")
                    for mtp in range(MT // 2):
                        nc.tensor.matmul(
                            qps,
                            wq8s[:, 2 * mtp : 2 * mtp + 2, et * P : (et + 1) * P],
                            hT8s[:, 2 * mtp : 2 * mtp + 2, :],
                            start=(mtp == 0),
                            stop=(mtp == MT // 2 - 1),
                            perf_mode=DR,
                        )
                    nc.vector.tensor_copy(qT[:, et, :], qps)

                # (emitted below, after pair-0 K proj: the DMA order delivers
                # wk8/cT8 before wq8/hT8)

                # ---- filler generators --------------------------------
                def emit_k_proj(hp, kT_t, tag="kps", tag2=None):
                    """K^T for pair hp: 4 jb blocks x 4 DR matmuls, yields
                    after each matmul / copy.  With tag2, j-block pairs share
                    each stationary load across two PSUM banks (the LDW is
                    the per-matmul bottleneck otherwise)."""
                    if tag2 is None:
                        for jb in range(4):
                            kps = psum.tile([P, I], F32, tag=tag, name="kps")
                            for mtp in range(MT // 2):
                                nc.tensor.matmul(
                                    kps,
                                    wk8s[:, 2 * mtp : 2 * mtp + 2,
                                         hp * P : (hp + 1) * P],
                                    cT8s[:, 2 * mtp : 2 * mtp + 2,
                                         jb * I : (jb + 1) * I],
                                    start=(mtp == 0),
                                    stop=(mtp == MT // 2 - 1),
                                    perf_mode=DR,
                                )
                                yield
                            nc.vector.tensor_copy(
                                kT_t[:, jb * I : (jb + 1) * I], kps)
                            yield
                        return
                    for jbp in range(2):
                        kps = [psum.tile([P, I], F32, tag=tg, name="kps")
                               for tg in (tag, tag2)]
                        for mtp in range(MT // 2):
                            w = wk8s[:, 2 * mtp : 2 * mtp + 2,
                                     hp * P : (hp + 1) * P]
                            for j2 in range(2):
                                jb = 2 * jbp + j2
                                nc.tensor.matmul(
                                    kps[j2], w,
                                    cT8s[:, 2 * mtp : 2 * mtp + 2,
                                         jb * I : (jb + 1) * I],
                                    start=(mtp == 0),
                                    stop=(mtp == MT // 2 - 1),
                                    perf_mode=DR,
                                )
                            yield
                        for j2 in range(2):
                            jb = 2 * jbp + j2
                            nc.vector.tensor_copy(
                                kT_t[:, jb * I : (jb + 1) * I], kps[j2])
                            yield

                def emit_v_proj():
                    """V projection sweep: one step per j-tile; each
                    stationary cT load is shared by both e-halves (two PSUM
                    banks), so a step computes all 16 heads' v~ columns for
                    that j-tile."""
                    for jt in range(JT):
                        vps = [psum.tile([P, I], F32, tag=tg, name="vps")
                               for tg in ("vps", "kps")]
                        for mtp in range(MT // 2):
                            cslice = cT8s[:, 2 * mtp : 2 * mtp + 2,
                                          jt * P : (jt + 1) * P]
                            for eh in range(2):
                                nc.tensor.matmul(
                                    vps[eh], cslice,
                                    wv8s[:, 2 * mtp : 2 * mtp + 2,
                                         eh * I : (eh + 1) * I],
                                    start=(mtp == 0),
                                    stop=(mtp == MT // 2 - 1),
                                    perf_mode=DR,
                                )
                        for eh in range(2):
                            nc.vector.tensor_copy(
                                v8[:, jt, eh * 8 : (eh + 1) * 8, 0:DH],
                                vps[eh].rearrange("p (h d) -> p h d", h=8),
                            )
                        yield

                # Head: K0, K1, q0, q1 all run in the input-DMA shadow.
                # K0/K1 ping-pong the two staging banks so their PSUM->SBUF
                # copies never stall the in-order PE stream.
                vgen = emit_v_proj()
                kT_tiles = {0: work.tile([P, J], BF16, tag="kT", name="kT0"),
                            1: work.tile([P, J], BF16, tag="kT", name="kT1")}
                for _ in emit_k_proj(0, kT_tiles[0], tag="kps", tag2="vps"):
                    pass
                for _ in emit_k_proj(1, kT_tiles[1], tag="kps", tag2="vps"):
                    pass
                q_step(0)
                q_step(1)
                kgen = iter(())

                exp = mybir.ActivationFunctionType.Exp
                av_tiles = {}
                p_tiles = {}

                def emit_bit_exp(eng, sc_ap, p_u8_ap, tag):
                    """exp(x)->fp8e4m3 off the ACT engine: the fp8 bits of
                    e^x are (to within mantissa interpolation) the integer
                    8*log2(e)*x + 56, so an affine map of the raw score plus
                    a saturating uint8 convert computes softmax numerators on
                    DVE/GPSIMD instead."""
                    AEXP = 11.5415603629
                    tmpe_t = work.tile([P, 2, I], F32, tag=tag, name="tmpe")
                    tmpe = tmpe_t.rearrange("p a b -> p (a b)")
                    eng.tensor_scalar(
                        tmpe,
                        sc_ap,
                        QK_SCALE * AEXP,
                        56.0 + EXP_SHIFT * AEXP,
                        op0=mybir.AluOpType.mult,
                        op1=mybir.AluOpType.add,
                    )
                    eng.tensor_scalar_max(p_u8_ap, tmpe, 0.0)

                def finalize_pair(hp):
                    """1/denominator, PE broadcast, write normalized av~."""
                    avs = av_tiles.pop(hp)
                    for hi in range(2):
                        av = avs[hi]
                        recip = work.tile([1, I], F32R, tag="recip",
                                          name="recip")
                        with nc.allow_low_precision(
                            reason="f32r keeps the f32 mantissa in SBUF"
                        ):
                            nc.vector.reciprocal(recip, av[DH : DH + 1, :])
                        rbc_ps = psum.tile([DH, I], F32, tag="kps",
                                           name="rbc_ps")
                        nc.tensor.matmul(rbc_ps, ones_row, recip,
                                         start=True, stop=True)
                        rbc = work.tile([DH, I], F32, tag="rbc", name="rbc")
                        nc.vector.tensor_copy(rbc, rbc_ps)
                        nc.vector.tensor_tensor(
                            avT[hi * DH : (hi + 1) * DH, hp, :],
                            av[0:DH, :],
                            rbc,
                            mybir.AluOpType.mult,
                        )

                # ---- flat chunk loop: 64 chunks + 1 epilogue ----------
                # Per-iteration emission order is chosen so the ACT exp
                # stream never starves: scores-A back-to-back (they only wait
                # on exp-A of the previous chunk, and run under exp-B), then
                # scores-B, then fillers, then the shifted attn@V.
                for t in range(NPAIR * NCH + 1):
                    hp, ci = divmod(t, NCH)
                    if t < NPAIR * NCH:
                        if ci == 0:
                            # fresh accumulators for this pair
                            av_tiles[hp] = (
                                psum.tile([P, I], F32, tag="avA", name="avA"),
                                psum.tile([P, I], F32, tag="avB", name="avB"),
                            )
                        kT_cur = kT_tiles[hp]
                        scA = psum.tile([P, 2, I], F32, tag="scA", name="scA")
                        scB = psum.tile([P, 2, I], F32, tag="scB", name="scB")
                        for k in range(2):
                            jt = 2 * ci + k
                            nc.tensor.matmul(
                                scA[:, k, :],
                                kT_cur[0:DH, jt * P : (jt + 1) * P],
                                qT[0:DH, hp, :],
                                start=True, stop=True,
                            )
                        pA = work.tile([P, 2, I], F8, tag="pA", name="pA")
                        nc.scalar.activation(
                            pA.rearrange("p a b -> p (a b)"),
                            scA.rearrange("p a b -> p (a b)"),
                            exp, scale=QK_SCALE, bias=shift_t,
                        )
                        for k in range(2):
                            jt = 2 * ci + k
                            nc.tensor.matmul(
                                scB[:, k, :],
                                kT_cur[DH:P, jt * P : (jt + 1) * P],
                                qT[DH:P, hp, :],
                                start=True, stop=True,
                            )
                        pB = work.tile([P, 2, I], F8, tag="pB", name="pB")
                        if hp >= 1 and ci in (2, 3, 5, 6):
                            # bit-exp the whole B half-chunk on the DVE
                            emit_bit_exp(
                                nc.vector,
                                scB.rearrange("p a b -> p (a b)"),
                                pB.bitcast(mybir.dt.uint8)
                                .rearrange("p a b -> p (a b)"),
                                tag="tmpeD",
                            )
                        else:
                            nc.scalar.activation(
                                pB.rearrange("p a b -> p (a b)"),
                                scB.rearrange("p a b -> p (a b)"),
                                exp, scale=QK_SCALE, bias=shift_t,
                            )
                        p_tiles[t] = (pA, pB)

                    # V-proj fillers: 2/chunk during pair 0 covers the whole
                    # sweep (each step now does both e-halves).
                    if t < NCH:
                        next(vgen, None)
                        next(vgen, None)

                    if t > 0:
                        hp_, cj = divmod(t - 1, NCH)
                        pA_, pB_ = p_tiles.pop(t - 1)
                        avs = av_tiles[hp_]
                        for hi, p_ in ((0, pA_), (1, pB_)):
                            nc.tensor.matmul(
                                avs[hi][0 : DH + 1, :],
                                v8[:, 2 * cj : 2 * cj + 2, 2 * hp_ + hi,
                                   0 : DH + 1],
                                p_[:, :, :],
                                start=(cj == 0),
                                stop=(cj == NCH - 1),
                                perf_mode=DR,
                            )
                        if cj == NCH - 1:
                            finalize_pair(hp_)

                    # pair-boundary work on the shared kps bank, in complete
                    # accumulation groups.  K0/K1 ran up front, so kgen for
                    # pair hp+1 only starts from pair 1 on; the deferred Q
                    # column goes one chunk later to keep boundaries light.
                    if t < NPAIR * NCH:
                        if ci == 0 and 1 <= hp < NPAIR - 1:
                            kT_tiles[hp + 1] = work.tile(
                                [P, J], BF16, tag="kT", name=f"kT{hp + 1}")
                            kgen = emit_k_proj(hp + 1, kT_tiles[hp + 1],
                                               tag="kps", tag2="vps")
                        if ci == 4 and 2 + (hp - 1) < ET and hp >= 1:
                            q_step(2 + (hp - 1))
                    for _ in range(2 if ci == 0 else 3):
                        next(kgen, None)

                # ---- O projection + residual + LN ---------------------
                # The residual is accumulated into the PSUM by an identity
                # matmul (Id.T @ hres = hres), so LN reads PSUM directly and
                # the DVE tail stays short.
                for it in range(I // P):
                    po = psum.tile([P, 2, I], F32,
                                   tag=("scA", "scB")[it % 2], name="po")
                    for etp in range(ET // 2):
                        lhs = avT[:, 2 * etp : 2 * etp + 2, it * P : (it + 1) * P]
                        for ob in range(2):
                            nc.tensor.matmul(
                                po[:, ob, :],
                                lhs,
                                wo8s[:, 2 * etp : 2 * etp + 2,
                                     ob * I : (ob + 1) * I],
                                start=(etp == 0),
                                stop=False,
                                perf_mode=DR,
                            )
                    for ob in range(2):
                        nc.tensor.matmul(
                            po[:, ob, :],
                            ident_t,
                            hres_ts[it][:, ob * I : (ob + 1) * I],
                            start=False,
                            stop=True,
                        )
                    stats = ph4.tile([P, 2, nc.vector.BN_STATS_DIM], F32,
                                     tag="stats")
                    for g in range(2):
                        nc.vector.bn_stats(stats[:, g, :], po[:, g, :])
                    mv = ph4.tile([P, nc.vector.BN_AGGR_DIM], F32, tag="mv")
                    nc.vector.bn_aggr(mv, stats)
                    rstd = ph4.tile([P, 1], F32, tag="rstd")
                    nc.scalar.activation(
                        rstd, mv[:, 1:2], mybir.ActivationFunctionType.Sqrt,
                        bias=eps_t,
                    )
                    nc.vector.reciprocal(rstd, rstd)
                    nmr = ph4.tile([P, 1], F32, tag="nmr")
                    nc.vector.tensor_tensor(nmr, mv[:, 0:1], rstd,
                                            mybir.AluOpType.mult)
                    nc.vector.tensor_scalar_mul(nmr, nmr, -1.0)
                    y = ph4.tile([P, D], F32, tag="y")
                    # (x - mu) * rstd on the (idle-in-tail) ACT engine
                    nc.scalar.activation(
                        y, po.rearrange("p a b -> p (a b)"),
                        mybir.ActivationFunctionType.Identity,
                        bias=nmr, scale=rstd,
                    )
                    if not trivial_ln_affine:
                        nc.vector.tensor_tensor(y, y, gamma_bc,
                                                mybir.AluOpType.mult)
                        nc.vector.tensor_tensor(y, y, beta_bc,
                                                mybir.AluOpType.add)
                    nc.sync.dma_start(out.ap()[it * P : (it + 1) * P, :], y)

    nc.compile()
    return nc


_NC_CACHE = {}


def _get_program(reps=1, trivial_ln_affine=False):
    key = (reps, trivial_ln_affine)
    if key not in _NC_CACHE:
        _NC_CACHE[key] = build_program(reps, trivial_ln_affine)
    return _NC_CACHE[key]


def _ln_affine_is_trivial(gamma, beta):
    return bool(np.all(np.asarray(gamma) == 1.0)
                and np.all(np.asarray(beta) == 0.0))


def _make_in_maps(h, c, Wq, Wkv, Wo, gamma, beta):
    import ml_dtypes

    F8NP = ml_dtypes.float8_e4m3

    h = np.asarray(h, dtype=np.float32)
    c = np.asarray(c, dtype=np.float32)
    Wq = np.asarray(Wq, dtype=np.float32)
    Wkv = np.asarray(Wkv, dtype=np.float32)
    Wo = np.asarray(Wo, dtype=np.float32)
    gamma = np.asarray(gamma, dtype=np.float32)
    beta = np.asarray(beta, dtype=np.float32)

    q_len, batch, d_model = h.shape
    assert (q_len, batch, d_model) == (1024, 4, D)

    def f8(x):
        return np.ascontiguousarray(np.asarray(x).astype(F8NP))

    wq8 = f8(WS * Wq.T)
    wk8 = f8(WS * Wkv[:D].T)
    wv8 = f8(WS * Wkv[D : 2 * D].T)
    wo8 = f8(WS * Wo.T)
    gamma_b = np.ascontiguousarray(np.broadcast_to(gamma, (P, D)))
    beta_b = np.ascontiguousarray(np.broadcast_to(beta, (P, D)))

    in_maps = []
    for core in range(8):
        b, g = divmod(core, 2)
        i0, i1 = g * I, (g + 1) * I
        in_maps.append({
            "hT8": f8(h[i0:i1, b, :].T),
            "cT8": f8(c[:, b, :].T),
            "wq8": wq8,
            "wk8": wk8,
            "wv8": wv8,
            "wo8": wo8,
            "hres": np.ascontiguousarray((RES_SCALE * h[i0:i1, b, :]).astype(ml_dtypes.bfloat16)),
            "ident": np.eye(P, dtype=ml_dtypes.bfloat16),
            "gamma": gamma_b,
            "beta": beta_b,
        })
    return in_maps


_RUNNERS = {}


def kernel(h, c, Wq, Wkv, Wo, gamma, beta):
    """Full-input entry point. The compiled executable is cached across
    calls so repeat invocations only pay transfer + execute."""
    in_maps = _make_in_maps(h, c, Wq, Wkv, Wo, gamma, beta)
    trivial = _ln_affine_is_trivial(gamma, beta)
    if trivial not in _RUNNERS:
        _RUNNERS[trivial] = _KernelRunner(
            _get_program(trivial_ln_affine=trivial))
    core_outs = _RUNNERS[trivial].run(in_maps)

    q_len, batch = 1024, 4
    out = np.empty((q_len, batch, D), dtype=np.float32)
    for core in range(8):
        b, g = divmod(core, 2)
        out[g * I : (g + 1) * I, b, :] = core_outs[core]["out"]
    return out


class _KernelRunner:
    """Persistent jitted SPMD executor (mirrors bass2jax.run_bass_via_pjrt,
    but reusable across calls with fresh inputs)."""

    def __init__(self, nc):
        import jax
        from jax.experimental.shard_map import shard_map
        from jax.sharding import Mesh, NamedSharding, PartitionSpec
        from concourse import bass2jax, mybir as _mybir

        bass2jax.install_neuronx_cc_hook()
        self._jax = jax
        partition_name = (nc.partition_id_tensor.name
                          if nc.partition_id_tensor else None)
        in_names, out_names, out_avals, zero_outs = [], [], [], []
        for alloc in nc.m.functions[0].allocations:
            if not isinstance(alloc, _mybir.MemoryLocationSet):
                continue
            name = alloc.memorylocations[0].name
            if alloc.kind == "ExternalInput":
                if name != partition_name:
                    in_names.append(name)
            elif alloc.kind == "ExternalOutput":
                shape = tuple(alloc.tensor_shape)
                dtype = _mybir.dt.np(alloc.dtype)
                out_names.append(name)
                out_avals.append(jax.core.ShapedArray(shape, dtype))
                zero_outs.append(np.zeros(shape, dtype))
        self._in_names, self._out_names = in_names, out_names
        self._out_avals, self._zero_outs = out_avals, zero_outs
        n_params = len(in_names)
        all_in = list(in_names) + list(out_names)
        if partition_name is not None:
            all_in.append(partition_name)

        def _body(*args):
            operands = list(args)
            if partition_name is not None:
                operands.append(bass2jax.partition_id_tensor())
            return tuple(bass2jax._bass_exec_p.bind(
                *operands, out_avals=tuple(out_avals),
                in_names=tuple(all_in), out_names=tuple(out_names),
                lowering_input_output_aliases=(),
                sim_require_finite=True, sim_require_nnan=True, nc=nc))

        donate = tuple(range(n_params, n_params + len(out_avals)))
        devices = jax.devices()[:8]
        mesh = Mesh(np.asarray(devices), ("core",))
        specs = (PartitionSpec("core"),)
        self._sharded = jax.jit(
            shard_map(_body, mesh=mesh,
                      in_specs=specs * (n_params + len(out_avals)),
                      out_specs=specs * len(out_avals), check_rep=False),
            donate_argnums=donate, keep_unused=True)
        self._sh = NamedSharding(mesh, PartitionSpec("core"))

    def run(self, in_maps):
        jax = self._jax
        dev_in = [jax.device_put(
            np.concatenate([np.asarray(in_maps[c][nm]) for c in range(8)],
                           axis=0), self._sh)
            for nm in self._in_names]
        zs = [jax.device_put(
            np.zeros((8 * z.shape[0], *z.shape[1:]), z.dtype), self._sh)
            for z in self._zero_outs]
        out_arrs = self._sharded(*dev_in, *zs)
        return [
            {name: np.asarray(out_arrs[i]).reshape(
                8, *self._out_avals[i].shape)[c]
             for i, name in enumerate(self._out_names)}
            for c in range(8)
        ]


def bench(inputs, iters=20, reps=1, chain=8):
    """Time the on-device execution: warm jit + pre-transferred inputs,
    chained-dispatch slope (cancels per-call overhead)."""
    import time

    import jax

    r = _BenchRunner(inputs, reps=reps)

    def run_chain(k):
        t = 0.0
        for _ in range(k):
            t += r.run()
        return t

    r.run(); r.run()
    slopes = []
    for _ in range(max(3, iters // 4)):
        t_a = run_chain(1)
        t_b = run_chain(chain)
        slopes.append((t_b - t_a) / (chain - 1.0))
    slopes.sort()
    med = slopes[len(slopes) // 2]
    print(f"bench(reps={reps}): slopes(us) = "
          f"{[f'{s*1e6:.0f}' for s in slopes]} -> median {med*1e6:.0f}us "
          f"min {slopes[0]*1e6:.0f}us")
    return med * 1e9


def bench_paired(inputs, pairs=10, hi_reps=8):
    """Paired-difference timing: interleave isolated calls of the reps=1 and
    reps=hi NEFFs; median of (t_hi - t_lo)/(hi-1) cancels slow drift."""
    r_lo = _BenchRunner(inputs, reps=1)
    r_hi = _BenchRunner(inputs, reps=hi_reps)
    r_lo.run(); r_hi.run(); r_lo.run(); r_hi.run()  # warm both
    diffs = []
    for _ in range(pairs):
        t_lo = r_lo.run()
        t_hi = r_hi.run()
        diffs.append((t_hi - t_lo) / (hi_reps - 1.0))
    diffs.sort()
    med = diffs[len(diffs) // 2]
    print(f"bench_paired: per-body diffs(us) = "
          f"{[f'{d*1e6:.0f}' for d in diffs]} -> median {med*1e6:.0f}us")
    return med * 1e9


class _BenchRunner:
    def __init__(self, inputs, reps):
        import jax
        from jax.experimental.shard_map import shard_map
        from jax.sharding import Mesh, NamedSharding, PartitionSpec
        from concourse import bass2jax, mybir as _mybir

        bass2jax.install_neuronx_cc_hook()
        nc = _get_program(reps, _ln_affine_is_trivial(inputs["gamma"],
                                                      inputs["beta"]))
        in_maps = _make_in_maps(**inputs)
        partition_name = (nc.partition_id_tensor.name
                          if nc.partition_id_tensor else None)
        in_names, out_names, out_avals, zero_outs = [], [], [], []
        for alloc in nc.m.functions[0].allocations:
            if not isinstance(alloc, _mybir.MemoryLocationSet):
                continue
            name = alloc.memorylocations[0].name
            if alloc.kind == "ExternalInput":
                if name != partition_name:
                    in_names.append(name)
            elif alloc.kind == "ExternalOutput":
                shape = tuple(alloc.tensor_shape)
                dtype = _mybir.dt.np(alloc.dtype)
                out_names.append(name)
                out_avals.append(jax.core.ShapedArray(shape, dtype))
                zero_outs.append(np.zeros(shape, dtype))
        n_params = len(in_names)
        all_in = list(in_names) + list(out_names)
        if partition_name is not None:
            all_in.append(partition_name)

        def _body(*args):
            operands = list(args)
            if partition_name is not None:
                operands.append(bass2jax.partition_id_tensor())
            return tuple(bass2jax._bass_exec_p.bind(
                *operands, out_avals=tuple(out_avals), in_names=tuple(all_in),
                out_names=tuple(out_names), lowering_input_output_aliases=(),
                sim_require_finite=True, sim_require_nnan=True, nc=nc))

        donate = tuple(range(n_params, n_params + len(out_avals)))
        devices = jax.devices()[:8]
        mesh = Mesh(np.asarray(devices), ("core",))
        specs = (PartitionSpec("core"),)
        self._sharded = jax.jit(
            shard_map(_body, mesh=mesh,
                      in_specs=specs * (n_params + len(out_avals)),
                      out_specs=specs * len(out_avals), check_rep=False),
            donate_argnums=donate, keep_unused=True)
        sh = NamedSharding(mesh, PartitionSpec("core"))
        self._dev_in = [jax.device_put(
            np.concatenate([np.asarray(in_maps[c][nm]) for c in range(8)],
                           axis=0), sh)
            for nm in in_names]
        self._zero_outs = zero_outs
        self._sh = sh
        self._jax = jax

    def run(self):
        import time
        jax = self._jax
        zs = [jax.device_put(
            np.zeros((8 * z.shape[0], *z.shape[1:]), z.dtype), self._sh)
            for z in self._zero_outs]
        jax.block_until_ready(zs)
        t0 = time.perf_counter()
        out = self._sharded(*self._dev_in, *zs)
        jax.block_until_ready(out)
        return time.perf_counter() - t0


# revision 56
# speedup vs baseline: 1.5427x; 1.5427x over previous
"""Multi-head cross-attention (post-LN) Trainium2 Bass kernel.

Full inputs -> full outputs. Sharding: 8 cores = 4 batches x 2 query-row
halves (512 rows each).  Host pre-transposes h/c/weights, scales weights by
32 and casts to fp8e4m3 so every big matmul runs in fp8 DoubleRow mode
(2 MACs/cell/cycle).  Scale folding keeps everything consistent:

  wq,wk,wv,wo scaled x32  ->  q~ = 32q, k~ = 32k, s~ = 1024 s
  exp scale = SCALE/1024      (softmax invariant)
  v~ = 32v -> av~ = 32 attn_vec (normalized by the ones-column denominator)
  psum_O = av~ @ 32Wo = 1024 attn_out;  hres = 1024 h  (LN is scale-inv,
  eps scaled by 1024^2)

Per-core pipeline (flat 64-chunk software pipeline, ACT-bound):
  Q proj (fp8 DR) -> q~ bf16 [e,i]
  K proj per pair (fp8 DR, filler-interleaved) -> kT bf16 [dh,j]
  V proj (fp8 DR, filler-interleaved) -> v~ fp8 in SBUF [j, jt, head, 80]
      (ones column at d=64 gives softmax denominators for free)
  scores: bf16 K=64 row-packed pairs -> PSUM [j,i]; exp on ACT -> p fp8
  attn@V: fp8 DR over j-tile pairs, accumulate [65, i] (row 64 = denom)
  normalize via PE ones-broadcast of 1/denom; O proj fp8 DR; residual+LN
"""

import sys

for _p in ("/opt/trn_rl_repo", "/root/.axon_site/_ro/trn_rl_repo"):
    if _p not in sys.path:
        sys.path.append(_p)

import numpy as np

import concourse.bass as bass
import concourse.tile as tile
from concourse import bacc, mybir
from concourse.bass_utils import run_bass_kernel_spmd

P = 128
D = 1024          # d_model
I = 512           # query rows per core
J = 2048          # kv length
NH = 16           # heads
DH = 64           # head dim
DHP = 80          # padded per-head stride in the SBUF V tile (16B aligned)
SCALE = 1.0 / (DH ** 0.5)
WS = 32.0         # weight pre-scale (weights ~N(0, 1/32) -> ~N(0,1) in fp8)
QK_SCALE = SCALE / (WS * WS)
EXP_SHIFT = -4.0  # global exp shift: keeps p in fp8e4m3 range (max ~150);
                  # softmax-invariant because the denominator uses the same p
RES_SCALE = WS * WS                      # hres = 1024*h host-side
EPS_ADJ = 1e-5 * RES_SCALE * RES_SCALE   # LN eps in the scaled domain
F32 = mybir.dt.float32
F32R = mybir.dt.float32r
BF16 = mybir.dt.bfloat16
F8 = mybir.dt.float8e4
DR = mybir.MatmulPerfMode.DoubleRow

MT = D // P       # 8 m-tiles (contraction over d_model)
ET = D // P       # 8 e-tiles (head features)
JT = J // P       # 16 j-tiles
NPAIR = NH // 2   # 8 head pairs
NCH = 8           # chunks per pair (2 j-tiles each)


def build_program(reps=1, trivial_ln_affine=False):
    nc = bacc.Bacc(None, target_bir_lowering=False, debug=False)

    hT8 = nc.dram_tensor("hT8", [D, I], F8, kind="ExternalInput")
    cT8 = nc.dram_tensor("cT8", [D, J], F8, kind="ExternalInput")
    wq8 = nc.dram_tensor("wq8", [D, D], F8, kind="ExternalInput")
    wk8 = nc.dram_tensor("wk8", [D, D], F8, kind="ExternalInput")
    wv8 = nc.dram_tensor("wv8", [D, D], F8, kind="ExternalInput")
    wo8 = nc.dram_tensor("wo8", [D, D], F8, kind="ExternalInput")
    hres = nc.dram_tensor("hres", [I, D], BF16, kind="ExternalInput")
    ident = nc.dram_tensor("ident", [P, P], BF16, kind="ExternalInput")
    gamma = nc.dram_tensor("gamma", [P, D], F32, kind="ExternalInput")
    beta = nc.dram_tensor("beta", [P, D], F32, kind="ExternalInput")
    out = nc.dram_tensor("out", [I, D], F32, kind="ExternalOutput")

    with tile.TileContext(nc) as tc:
        with (
            tc.tile_pool(name="consts", bufs=1) as consts,
            tc.tile_pool(name="persist", bufs=1) as persist,
            tc.tile_pool(name="work", bufs=2) as work,
            tc.tile_pool(name="ph4", bufs=2) as ph4,
            tc.tile_pool(name="psum", bufs=1, space="PSUM") as psum,
        ):
            gamma_bc = consts.tile([P, D], F32, tag="gamma_bc")
            beta_bc = consts.tile([P, D], F32, tag="beta_bc")
            ones_row = consts.tile([1, DH], F32R, tag="ones_row")
            nc.vector.memset(ones_row.bitcast(F32), 1.0)
            eps_t = consts.tile([P, 1], F32, tag="eps")
            nc.vector.memset(eps_t, EPS_ADJ)
            shift_t = consts.tile([P, 1], F32, tag="shift")
            nc.vector.memset(shift_t, EXP_SHIFT)

            hT8s = persist.tile([P, MT, I], F8, tag="hT8s")
            cT8s = persist.tile([P, MT, J], F8, tag="cT8s")
            wq8s = persist.tile([P, MT, D], F8, tag="wq8s")
            wk8s = persist.tile([P, MT, D], F8, tag="wk8s")
            wv8s = persist.tile([P, MT, D], F8, tag="wv8s")
            wo8s = persist.tile([P, ET, D], F8, tag="wo8s")
            qT = persist.tile([P, ET, I], BF16, tag="qT")
            v8 = persist.tile([P, JT, NH, DHP], F8, tag="v8")
            avT = persist.tile([P, ET, I], F8, tag="avT")

            # softmax-denominator ones column (constant across reps)
            nc.vector.memset(v8[:, :, :, DH : DH + 1], 1.0)

            for _rep in range(reps):
                # ---- input DMA: one big transfer per tensor (the per-call
                # issue overhead is ~1.2us, so small per-slice DMAs would
                # serialize into a huge head), K-path inputs first ---------
                nc.sync.dma_start(
                    wk8s[:, :, :],
                    wk8.ap().rearrange("(mt p) e -> p mt e", p=P))
                nc.scalar.dma_start(
                    cT8s[:, :, 0 : J // 2],
                    cT8.ap()[:, 0 : J // 2]
                    .rearrange("(mt p) j -> p mt j", p=P))
                nc.scalar.dma_start(
                    cT8s[:, :, J // 2 : J],
                    cT8.ap()[:, J // 2 : J]
                    .rearrange("(mt p) j -> p mt j", p=P))
                nc.sync.dma_start(
                    wq8s[:, :, :],
                    wq8.ap().rearrange("(mt p) e -> p mt e", p=P))
                nc.sync.dma_start(
                    hT8s[:, :, :],
                    hT8.ap().rearrange("(mt p) i -> p mt i", p=P))
                nc.sync.dma_start(
                    wv8s[:, :, :],
                    wv8.ap().rearrange("(mt p) e -> p mt e", p=P))
                nc.sync.dma_start(
                    wo8s[:, :, :],
                    wo8.ap().rearrange("(et p) o -> p et o", p=P))
                hres_all = ph4.tile([P, I // P, D], BF16, tag="hres", bufs=1)
                nc.sync.dma_start(
                    hres_all,
                    hres.ap().rearrange("(it p) d -> p it d", p=P))
                hres_ts = [hres_all[:, it, :] for it in range(I // P)]
                ident_t = consts.tile([P, P], BF16, tag="ident")
                nc.sync.dma_start(ident_t, ident.ap())
                if not trivial_ln_affine:
                    nc.sync.dma_start(gamma_bc, gamma.ap())
                    nc.sync.dma_start(beta_bc, beta.ap())

                # ---- Q projection (fp8 DoubleRow) ---------------------
                def q_step(et):
                    qps = psum.tile([P, I], F32, tag="kps", name="qps")
                    for mtp in range(MT // 2):
                        nc.tensor.matmul(
                            qps,
                            wq8s[:, 2 * mtp : 2 * mtp + 2, et * P : (et + 1) * P],
                            hT8s[:, 2 * mtp : 2 * mtp + 2, :],
                            start=(mtp == 0),
                            stop=(mtp == MT // 2 - 1),
                            perf_mode=DR,
                        )
                    nc.vector.tensor_copy(qT[:, et, :], qps)

                # (emitted below, after pair-0 K proj: the DMA order delivers
                # wk8/cT8 before wq8/hT8)

                # ---- filler generators --------------------------------
                def emit_k_proj(hp, kT_t, tag="kps", tag2=None):
                    """K^T for pair hp: 4 jb blocks x 4 DR matmuls, yields
                    after each matmul / copy.  With tag2, j-block pairs share
                    each stationary load across two PSUM banks (the LDW is
                    the per-matmul bottleneck otherwise)."""
                    if tag2 is None:
                        for jb in range(4):
                            kps = psum.tile([P, I], F32, tag=tag, name="kps")
                            for mtp in range(MT // 2):
                                nc.tensor.matmul(
                                    kps,
                                    wk8s[:, 2 * mtp : 2 * mtp + 2,
                                         hp * P : (hp + 1) * P],
                                    cT8s[:, 2 * mtp : 2 * mtp + 2,
                                         jb * I : (jb + 1) * I],
                                    start=(mtp == 0),
                                    stop=(mtp == MT // 2 - 1),
                                    perf_mode=DR,
                                )
                                yield
                            nc.vector.tensor_copy(
                                kT_t[:, jb * I : (jb + 1) * I], kps)
                            yield
                        return
                    for jbp in range(2):
                        kps = [psum.tile([P, I], F32, tag=tg, name="kps")
                               for tg in (tag, tag2)]
                        for mtp in range(MT // 2):
                            w = wk8s[:, 2 * mtp : 2 * mtp + 2,
                                     hp * P : (hp + 1) * P]
                            for j2 in range(2):
                                jb = 2 * jbp + j2
                                nc.tensor.matmul(
                                    kps[j2], w,
                                    cT8s[:, 2 * mtp : 2 * mtp + 2,
                                         jb * I : (jb + 1) * I],
                                    start=(mtp == 0),
                                    stop=(mtp == MT // 2 - 1),
                                    perf_mode=DR,
                                )
                            yield
                        for j2 in range(2):
                            jb = 2 * jbp + j2
                            nc.vector.tensor_copy(
                                kT_t[:, jb * I : (jb + 1) * I], kps[j2])
                            yield

                def emit_v_proj():
                    """V projection sweep: one step per j-tile; each
                    stationary cT load is shared by both e-halves (two PSUM
                    banks), so a step computes all 16 heads' v~ columns for
                    that j-tile."""
                    for jt in range(JT):
                        vps = [psum.tile([P, I], F32, tag=tg, name="vps")
                               for tg in ("vps", "kps")]
                        for mtp in range(MT // 2):
                            cslice = cT8s[:, 2 * mtp : 2 * mtp + 2,
                                          jt * P : (jt + 1) * P]
                            for eh in range(2):
                                nc.tensor.matmul(
                                    vps[eh], cslice,
                                    wv8s[:, 2 * mtp : 2 * mtp + 2,
                                         eh * I : (eh + 1) * I],
                                    start=(mtp == 0),
                                    stop=(mtp == MT // 2 - 1),
                                    perf_mode=DR,
                                )
                        for eh in range(2):
                            nc.vector.tensor_copy(
                                v8[:, jt, eh * 8 : (eh + 1) * 8, 0:DH],
                                vps[eh].rearrange("p (h d) -> p h d", h=8),
                            )
                        yield

                # Head: K0, K1, q0, q1 all run in the input-DMA shadow.
                # K0/K1 ping-pong the two staging banks so their PSUM->SBUF
                # copies never stall the in-order PE stream.
                vgen = emit_v_proj()
                kT_tiles = {0: work.tile([P, J], BF16, tag="kT", name="kT0"),
                            1: work.tile([P, J], BF16, tag="kT", name="kT1")}
                for _ in emit_k_proj(0, kT_tiles[0], tag="kps", tag2="vps"):
                    pass
                for _ in emit_k_proj(1, kT_tiles[1], tag="kps", tag2="vps"):
                    pass
                q_step(0)
                q_step(1)
                kgen = iter(())

                exp = mybir.ActivationFunctionType.Exp
                av_tiles = {}
                p_tiles = {}

                def finalize_pair(hp):
                    """1/denominator, PE broadcast, write normalized av~."""
                    avs = av_tiles.pop(hp)
                    for hi in range(2):
                        av = avs[hi]
                        recip = work.tile([1, I], F32R, tag="recip",
                                          name="recip")
                        with nc.allow_low_precision(
                            reason="f32r keeps the f32 mantissa in SBUF"
                        ):
                            nc.vector.reciprocal(recip, av[DH : DH + 1, :])
                        rbc_ps = psum.tile([DH, I], F32, tag="kps",
                                           name="rbc_ps")
                        nc.tensor.matmul(rbc_ps, ones_row, recip,
                                         start=True, stop=True)
                        rbc = work.tile([DH, I], F32, tag="rbc", name="rbc")
                        nc.vector.tensor_copy(rbc, rbc_ps)
                        nc.vector.tensor_tensor(
                            avT[hi * DH : (hi + 1) * DH, hp, :],
                            av[0:DH, :],
                            rbc,
                            mybir.AluOpType.mult,
                        )

                # ---- flat chunk loop: 64 chunks + 1 epilogue ----------
                # Per-iteration emission order is chosen so the ACT exp
                # stream never starves: scores-A back-to-back (they only wait
                # on exp-A of the previous chunk, and run under exp-B), then
                # scores-B, then fillers, then the shifted attn@V.
                for t in range(NPAIR * NCH + 1):
                    hp, ci = divmod(t, NCH)
                    if t < NPAIR * NCH:
                        if ci == 0:
                            # fresh accumulators for this pair
                            av_tiles[hp] = (
                                psum.tile([P, I], F32, tag="avA", name="avA"),
                                psum.tile([P, I], F32, tag="avB", name="avB"),
                            )
                        kT_cur = kT_tiles[hp]
                        scA = psum.tile([P, 2, I], F32, tag="scA", name="scA")
                        scB = psum.tile([P, 2, I], F32, tag="scB", name="scB")
                        for k in range(2):
                            jt = 2 * ci + k
                            nc.tensor.matmul(
                                scA[:, k, :],
                                kT_cur[0:DH, jt * P : (jt + 1) * P],
                                qT[0:DH, hp, :],
                                start=True, stop=True,
                            )
                        pA = work.tile([P, 2, I], F8, tag="pA", name="pA")
                        nc.scalar.activation(
                            pA.rearrange("p a b -> p (a b)"),
                            scA.rearrange("p a b -> p (a b)"),
                            exp, scale=QK_SCALE, bias=shift_t,
                        )
                        for k in range(2):
                            jt = 2 * ci + k
                            nc.tensor.matmul(
                                scB[:, k, :],
                                kT_cur[DH:P, jt * P : (jt + 1) * P],
                                qT[DH:P, hp, :],
                                start=True, stop=True,
                            )
                        pB = work.tile([P, 2, I], F8, tag="pB", name="pB")
                        nc.scalar.activation(
                            pB.rearrange("p a b -> p (a b)"),
                            scB.rearrange("p a b -> p (a b)"),
                            exp, scale=QK_SCALE, bias=shift_t,
                        )
                        p_tiles[t] = (pA, pB)

                    # V-proj fillers: 2/chunk during pair 0 covers the whole
                    # sweep (each step now does both e-halves).
                    if t < NCH:
                        next(vgen, None)
                        next(vgen, None)

                    if t > 0:
                        hp_, cj = divmod(t - 1, NCH)
                        pA_, pB_ = p_tiles.pop(t - 1)
                        avs = av_tiles[hp_]
                        for hi, p_ in ((0, pA_), (1, pB_)):
                            nc.tensor.matmul(
                                avs[hi][0 : DH + 1, :],
                                v8[:, 2 * cj : 2 * cj + 2, 2 * hp_ + hi,
                                   0 : DH + 1],
                                p_[:, :, :],
                                start=(cj == 0),
                                stop=(cj == NCH - 1),
                                perf_mode=DR,
                            )
                        if cj == NCH - 1:
                            finalize_pair(hp_)

                    # pair-boundary work on the shared kps bank, in complete
                    # accumulation groups.  K0/K1 ran up front, so kgen for
                    # pair hp+1 only starts from pair 1 on; the deferred Q
                    # column goes one chunk later to keep boundaries light.
                    if t < NPAIR * NCH:
                        if ci == 0 and 1 <= hp < NPAIR - 1:
                            kT_tiles[hp + 1] = work.tile(
                                [P, J], BF16, tag="kT", name=f"kT{hp + 1}")
                            kgen = emit_k_proj(hp + 1, kT_tiles[hp + 1],
                                               tag="kps", tag2="vps")
                        if ci == 4 and 2 + (hp - 1) < ET and hp >= 1:
                            q_step(2 + (hp - 1))
                    for _ in range(2 if ci == 0 else 3):
                        next(kgen, None)

                # ---- O projection + residual + LN ---------------------
                # The residual is accumulated into the PSUM by an identity
                # matmul (Id.T @ hres = hres), so LN reads PSUM directly and
                # the DVE tail stays short.
                for it in range(I // P):
                    po = psum.tile([P, 2, I], F32,
                                   tag=("scA", "scB")[it % 2], name="po")
                    for etp in range(ET // 2):
                        lhs = avT[:, 2 * etp : 2 * etp + 2, it * P : (it + 1) * P]
                        for ob in range(2):
                            nc.tensor.matmul(
                                po[:, ob, :],
                                lhs,
                                wo8s[:, 2 * etp : 2 * etp + 2,
                                     ob * I : (ob + 1) * I],
                                start=(etp == 0),
                                stop=False,
                                perf_mode=DR,
                            )
                    for ob in range(2):
                        nc.tensor.matmul(
                            po[:, ob, :],
                            ident_t,
                            hres_ts[it][:, ob * I : (ob + 1) * I],
                            start=False,
                            stop=True,
                        )
                    stats = ph4.tile([P, 2, nc.vector.BN_STATS_DIM], F32,
                                     tag="stats")
                    for g in range(2):
                        nc.vector.bn_stats(stats[:, g, :], po[:, g, :])
                    mv = ph4.tile([P, nc.vector.BN_AGGR_DIM], F32, tag="mv")
                    nc.vector.bn_aggr(mv, stats)
                    rstd = ph4.tile([P, 1], F32, tag="rstd")
                    nc.scalar.activation(
                        rstd, mv[:, 1:2], mybir.ActivationFunctionType.Sqrt,
                        bias=eps_t,
                    )
                    nc.vector.reciprocal(rstd, rstd)
                    nmr = ph4.tile([P, 1], F32, tag="nmr")
                    nc.vector.tensor_tensor(nmr, mv[:, 0:1], rstd,
                                            mybir.AluOpType.mult)
                    nc.vector.tensor_scalar_mul(nmr, nmr, -1.0)
                    y = ph4.tile([P, D], F32, tag="y")
                    # (x - mu) * rstd on the (idle-in-tail) ACT engine
                    nc.scalar.activation(
                        y, po.rearrange("p a b -> p (a b)"),
                        mybir.ActivationFunctionType.Identity,
                        bias=nmr, scale=rstd,
                    )
                    if not trivial_ln_affine:
                        nc.vector.tensor_tensor(y, y, gamma_bc,
                                                mybir.AluOpType.mult)
                        nc.vector.tensor_tensor(y, y, beta_bc,
                                                mybir.AluOpType.add)
                    nc.sync.dma_start(out.ap()[it * P : (it + 1) * P, :], y)

    nc.compile()
    return nc


_NC_CACHE = {}


def _get_program(reps=1, trivial_ln_affine=False):
    key = (reps, trivial_ln_affine)
    if key not in _NC_CACHE:
        _NC_CACHE[key] = build_program(reps, trivial_ln_affine)
    return _NC_CACHE[key]


def _ln_affine_is_trivial(gamma, beta):
    return bool(np.all(np.asarray(gamma) == 1.0)
                and np.all(np.asarray(beta) == 0.0))


def _make_in_maps(h, c, Wq, Wkv, Wo, gamma, beta):
    import ml_dtypes

    F8NP = ml_dtypes.float8_e4m3

    h = np.asarray(h, dtype=np.float32)
    c = np.asarray(c, dtype=np.float32)
    Wq = np.asarray(Wq, dtype=np.float32)
    Wkv = np.asarray(Wkv, dtype=np.float32)
    Wo = np.asarray(Wo, dtype=np.float32)
    gamma = np.asarray(gamma, dtype=np.float32)
    beta = np.asarray(beta, dtype=np.float32)

    q_len, batch, d_model = h.shape
    assert (q_len, batch, d_model) == (1024, 4, D)

    def f8(x):
        return np.ascontiguousarray(np.asarray(x).astype(F8NP))

    wq8 = f8(WS * Wq.T)
    wk8 = f8(WS * Wkv[:D].T)
    wv8 = f8(WS * Wkv[D : 2 * D].T)
    wo8 = f8(WS * Wo.T)
    gamma_b = np.ascontiguousarray(np.broadcast_to(gamma, (P, D)))
    beta_b = np.ascontiguousarray(np.broadcast_to(beta, (P, D)))

    in_maps = []
    for core in range(8):
        b, g = divmod(core, 2)
        i0, i1 = g * I, (g + 1) * I
        in_maps.append({
            "hT8": f8(h[i0:i1, b, :].T),
            "cT8": f8(c[:, b, :].T),
            "wq8": wq8,
            "wk8": wk8,
            "wv8": wv8,
            "wo8": wo8,
            "hres": np.ascontiguousarray((RES_SCALE * h[i0:i1, b, :]).astype(ml_dtypes.bfloat16)),
            "ident": np.eye(P, dtype=ml_dtypes.bfloat16),
            "gamma": gamma_b,
            "beta": beta_b,
        })
    return in_maps


_RUNNERS = {}


def kernel(h, c, Wq, Wkv, Wo, gamma, beta):
    """Full-input entry point. The compiled executable is cached across
    calls so repeat invocations only pay transfer + execute."""
    in_maps = _make_in_maps(h, c, Wq, Wkv, Wo, gamma, beta)
    trivial = _ln_affine_is_trivial(gamma, beta)
    if trivial not in _RUNNERS:
        _RUNNERS[trivial] = _KernelRunner(
            _get_program(trivial_ln_affine=trivial))
    core_outs = _RUNNERS[trivial].run(in_maps)

    q_len, batch = 1024, 4
    out = np.empty((q_len, batch, D), dtype=np.float32)
    for core in range(8):
        b, g = divmod(core, 2)
        out[g * I : (g + 1) * I, b, :] = core_outs[core]["out"]
    return out


class _KernelRunner:
    """Persistent jitted SPMD executor (mirrors bass2jax.run_bass_via_pjrt,
    but reusable across calls with fresh inputs)."""

    def __init__(self, nc):
        import jax
        from jax.experimental.shard_map import shard_map
        from jax.sharding import Mesh, NamedSharding, PartitionSpec
        from concourse import bass2jax, mybir as _mybir

        bass2jax.install_neuronx_cc_hook()
        self._jax = jax
        partition_name = (nc.partition_id_tensor.name
                          if nc.partition_id_tensor else None)
        in_names, out_names, out_avals, zero_outs = [], [], [], []
        for alloc in nc.m.functions[0].allocations:
            if not isinstance(alloc, _mybir.MemoryLocationSet):
                continue
            name = alloc.memorylocations[0].name
            if alloc.kind == "ExternalInput":
                if name != partition_name:
                    in_names.append(name)
            elif alloc.kind == "ExternalOutput":
                shape = tuple(alloc.tensor_shape)
                dtype = _mybir.dt.np(alloc.dtype)
                out_names.append(name)
                out_avals.append(jax.core.ShapedArray(shape, dtype))
                zero_outs.append(np.zeros(shape, dtype))
        self._in_names, self._out_names = in_names, out_names
        self._out_avals, self._zero_outs = out_avals, zero_outs
        n_params = len(in_names)
        all_in = list(in_names) + list(out_names)
        if partition_name is not None:
            all_in.append(partition_name)

        def _body(*args):
            operands = list(args)
            if partition_name is not None:
                operands.append(bass2jax.partition_id_tensor())
            return tuple(bass2jax._bass_exec_p.bind(
                *operands, out_avals=tuple(out_avals),
                in_names=tuple(all_in), out_names=tuple(out_names),
                lowering_input_output_aliases=(),
                sim_require_finite=True, sim_require_nnan=True, nc=nc))

        donate = tuple(range(n_params, n_params + len(out_avals)))
        devices = jax.devices()[:8]
        mesh = Mesh(np.asarray(devices), ("core",))
        specs = (PartitionSpec("core"),)
        self._sharded = jax.jit(
            shard_map(_body, mesh=mesh,
                      in_specs=specs * (n_params + len(out_avals)),
                      out_specs=specs * len(out_avals), check_rep=False),
            donate_argnums=donate, keep_unused=True)
        self._sh = NamedSharding(mesh, PartitionSpec("core"))

    def run(self, in_maps):
        jax = self._jax
        dev_in = [jax.device_put(
            np.concatenate([np.asarray(in_maps[c][nm]) for c in range(8)],
                           axis=0), self._sh)
            for nm in self._in_names]
        zs = [jax.device_put(
            np.zeros((8 * z.shape[0], *z.shape[1:]), z.dtype), self._sh)
            for z in self._zero_outs]
        out_arrs = self._sharded(*dev_in, *zs)
        return [
            {name: np.asarray(out_arrs[i]).reshape(
                8, *self._out_avals[i].shape)[c]
             for i, name in enumerate(self._out_names)}
            for c in range(8)
        ]


def bench(inputs, iters=20, reps=1, chain=8):
    """Time the on-device execution: warm jit + pre-transferred inputs,
    chained-dispatch slope (cancels per-call overhead)."""
    import time

    import jax

    r = _BenchRunner(inputs, reps=reps)

    def run_chain(k):
        t = 0.0
        for _ in range(k):
            t += r.run()
        return t

    r.run(); r.run()
    slopes = []
    for _ in range(max(3, iters // 4)):
        t_a = run_chain(1)
        t_b = run_chain(chain)
        slopes.append((t_b - t_a) / (chain - 1.0))
    slopes.sort()
    med = slopes[len(slopes) // 2]
    print(f"bench(reps={reps}): slopes(us) = "
          f"{[f'{s*1e6:.0f}' for s in slopes]} -> median {med*1e6:.0f}us "
          f"min {slopes[0]*1e6:.0f}us")
    return med * 1e9


def bench_paired(inputs, pairs=10, hi_reps=8):
    """Paired-difference timing: interleave isolated calls of the reps=1 and
    reps=hi NEFFs; median of (t_hi - t_lo)/(hi-1) cancels slow drift."""
    r_lo = _BenchRunner(inputs, reps=1)
    r_hi = _BenchRunner(inputs, reps=hi_reps)
    r_lo.run(); r_hi.run(); r_lo.run(); r_hi.run()  # warm both
    diffs = []
    for _ in range(pairs):
        t_lo = r_lo.run()
        t_hi = r_hi.run()
        diffs.append((t_hi - t_lo) / (hi_reps - 1.0))
    diffs.sort()
    med = diffs[len(diffs) // 2]
    print(f"bench_paired: per-body diffs(us) = "
          f"{[f'{d*1e6:.0f}' for d in diffs]} -> median {med*1e6:.0f}us")
    return med * 1e9


class _BenchRunner:
    def __init__(self, inputs, reps):
        import jax
        from jax.experimental.shard_map import shard_map
        from jax.sharding import Mesh, NamedSharding, PartitionSpec
        from concourse import bass2jax, mybir as _mybir

        bass2jax.install_neuronx_cc_hook()
        nc = _get_program(reps, _ln_affine_is_trivial(inputs["gamma"],
                                                      inputs["beta"]))
        in_maps = _make_in_maps(**inputs)
        partition_name = (nc.partition_id_tensor.name
                          if nc.partition_id_tensor else None)
        in_names, out_names, out_avals, zero_outs = [], [], [], []
        for alloc in nc.m.functions[0].allocations:
            if not isinstance(alloc, _mybir.MemoryLocationSet):
                continue
            name = alloc.memorylocations[0].name
            if alloc.kind == "ExternalInput":
                if name != partition_name:
                    in_names.append(name)
            elif alloc.kind == "ExternalOutput":
                shape = tuple(alloc.tensor_shape)
                dtype = _mybir.dt.np(alloc.dtype)
                out_names.append(name)
                out_avals.append(jax.core.ShapedArray(shape, dtype))
                zero_outs.append(np.zeros(shape, dtype))
        n_params = len(in_names)
        all_in = list(in_names) + list(out_names)
        if partition_name is not None:
            all_in.append(partition_name)

        def _body(*args):
            operands = list(args)
            if partition_name is not None:
                operands.append(bass2jax.partition_id_tensor())
            return tuple(bass2jax._bass_exec_p.bind(
                *operands, out_avals=tuple(out_avals), in_names=tuple(all_in),
                out_names=tuple(out_names), lowering_input_output_aliases=(),
                sim_require_finite=True, sim_require_nnan=True, nc=nc))

        donate = tuple(range(n_params, n_params + len(out_avals)))
        devices = jax.devices()[:8]
        mesh = Mesh(np.asarray(devices), ("core",))
        specs = (PartitionSpec("core"),)
        self._sharded = jax.jit(
            shard_map(_body, mesh=mesh,
                      in_specs=specs * (n_params + len(out_avals)),
                      out_specs=specs * len(out_avals), check_rep=False),
            donate_argnums=donate, keep_unused=True)
        sh = NamedSharding(mesh, PartitionSpec("core"))
        self._dev_in = [jax.device_put(
            np.concatenate([np.asarray(in_maps[c][nm]) for c in range(8)],
                           axis=0), sh)
            for nm in in_names]
        self._zero_outs = zero_outs
        self._sh = sh
        self._jax = jax

    def run(self):
        import time
        jax = self._jax
        zs = [jax.device_put(
            np.zeros((8 * z.shape[0], *z.shape[1:]), z.dtype), self._sh)
            for z in self._zero_outs]
        jax.block_until_ready(zs)
        t0 = time.perf_counter()
        out = self._sharded(*self._dev_in, *zs)
        jax.block_until_ready(out)
        return time.perf_counter() - t0
